# revision 23
# baseline (speedup 1.0000x reference)
"""Multi-head attention (b=2, l=2048, d_model=1024, h=16) on 8 trn2 NeuronCores.

Sharding: tensor-parallel over heads. Each core owns 2 heads (128 qkv
channels): it computes its QKV projections, attention for its heads, and a
rank-128 partial of the output projection. The host sums the 8 bf16 partials
and adds b_o (the tensor-parallel all-reduce, done at gather time).

v2 design (ACT-paced): all matmul operands bf16 (fp32 psum accumulate).
  phase A: QT/KT [128ch, 4096tok] = W.T @ xT streamed per 512-token chunk;
           V produced directly in natural layout [tok, 130] via xT-stationary
           matmuls against Wv_aug = [V_h0 | 0 | V_h1 | 0] plus a ones-row
           bias matmul that also bakes the softmax ones-columns.
  attention per (b, 512-q-chunk, k-tile): one [128,1024] psum tile holds both
           heads' scoresT (row-group-packed concurrent matmuls); one N=1024
           exp on ACT (the pacer); PV per head accumulates [V_h|1].T @ exp
           into [65, 512] psum over 16 k-tiles (row 64 = softmax denom Z).
  evac per (b, qc): reciprocal(Z) -> gpsimd partition_broadcast -> fused
           multiply: attnU is stored PRE-normalized (bf16), so the output
           projection is a single 128-contraction matmul per [128tok, 512]
           unit, evacuated bf16 and DMA'd out.
  Emission is software-pipelined: phase-A b1 chunks and out-projection units
  are interleaved into the kt-unit stream to fill PE slack under ACT.
"""
import sys
import types

import numpy as np

D_MODEL = 1024
H = 16
DH = 64
B = 2
L = 2048
BL = B * L            # 4096 tokens
NCORES = 8
NKT = D_MODEL // 128  # 8 dmodel tiles
TCH = 512             # phase-A token chunk
NCH = BL // TCH       # 8 chunks
QC = 512              # attention q chunk (per head)
NQC = L // QC         # 4 per batch
NKB = L // 128        # 16 k-tiles per batch
VW = 2 * (DH + 1)     # 130: [V_h0 | 1 | V_h1 | 1]


def _register_ntff_hook():
    """Install the axon NTFF profiling hook module if the image lacks it."""
    if "antenv.axon_hooks" in sys.modules:
        return
    try:
        import antenv
        mod = types.ModuleType("antenv.axon_hooks")
        holder = {}
        mod.set_axon_ntff_profile_hook = lambda h: holder.__setitem__("h", h)
        mod.get_axon_ntff_profile_hook = lambda: holder.get("h")
        sys.modules["antenv.axon_hooks"] = mod
        antenv.axon_hooks = mod
        from trn_agent_boot.trn_boot import _ntff_profile_via_ctypes
        mod.set_axon_ntff_profile_hook(
            _ntff_profile_via_ctypes("/opt/axon/libaxon_pjrt.so")
        )
    except Exception:
        pass


_NC_CACHE = {}


def _build():
    if "nc" in _NC_CACHE:
        return _NC_CACHE["nc"]
    import concourse.bacc as bacc
    import concourse.tile as tile
    import concourse.mybir as mybir
    from concourse.bass import AP

    F32 = mybir.dt.float32
    BF16 = mybir.dt.bfloat16
    AF = mybir.ActivationFunctionType
    ALU = mybir.AluOpType

    nc = bacc.Bacc("TRN2", target_bir_lowering=False, debug=False)

    xT_d = nc.dram_tensor("xT", [D_MODEL, BL], BF16, kind="ExternalInput").ap()
    wq_d = nc.dram_tensor("wq", [128, NKT * 128], BF16, kind="ExternalInput").ap()
    wk_d = nc.dram_tensor("wk", [128, NKT * 128], BF16, kind="ExternalInput").ap()
    wv_d = nc.dram_tensor("wv", [128, NKT, VW], BF16, kind="ExternalInput").ap()
    bq_d = nc.dram_tensor("bq", [128, 1], F32, kind="ExternalInput").ap()
    bk_d = nc.dram_tensor("bk", [128, 1], F32, kind="ExternalInput").ap()
    bv_d = nc.dram_tensor("bv", [1, VW], BF16, kind="ExternalInput").ap()
    wo_d = nc.dram_tensor("wo", [128, D_MODEL], BF16, kind="ExternalInput").ap()
    out_d = nc.dram_tensor("out", [BL, D_MODEL], BF16, kind="ExternalOutput").ap()

    with tile.TileContext(nc) as tc:
        with (
            tc.tile_pool(name="weights", bufs=1) as wpool,
            tc.tile_pool(name="persist", bufs=1) as ppool,
            tc.tile_pool(name="xin", bufs=NCH) as xpool,
            tc.tile_pool(name="expP", bufs=3) as epool,
            tc.tile_pool(name="oout", bufs=3) as opool,
            tc.tile_pool(name="rzP", bufs=2) as rzpool,
            tc.tile_pool(name="zrP", bufs=2) as zrpool,
            tc.tile_pool(name="dram", bufs=2, space="DRAM") as dpool,
            tc.tile_pool(name="psS", bufs=2, space="PSUM") as psS,
            tc.tile_pool(name="psPV", bufs=1, space="PSUM") as psPV,
            tc.tile_pool(name="psG", bufs=2, space="PSUM") as psG,
        ):
            # ---- static tiles ----
            wq_t = wpool.tile([128, NKT * 128], BF16, tag="wq")
            wk_t = wpool.tile([128, NKT * 128], BF16, tag="wk")
            wv_t = wpool.tile([128, NKT, VW], BF16, tag="wv")
            bq_t = wpool.tile([128, 1], F32, tag="bq")
            bk_t = wpool.tile([128, 1], F32, tag="bk")
            bv_t = wpool.tile([1, VW], BF16, tag="bv")
            wo_t = wpool.tile([128, D_MODEL], BF16, tag="wo")
            for t, d in ((wq_t, wq_d), (wk_t, wk_d), (wv_t, wv_d),
                         (bq_t, bq_d), (bk_t, bk_d), (bv_t, bv_d),
                         (wo_t, wo_d)):
                nc.gpsimd.dma_start(t[:], d)

            QT = ppool.tile([128, BL], BF16, tag="QT")
            KT = ppool.tile([128, BL], BF16, tag="KT")
            Vaug = ppool.tile([128, B * NKB, VW], BF16, tag="Vaug")
            attnU = [ppool.tile([128, L], BF16, tag=f"attnU{b}",
                                name=f"attnU{b}") for b in range(B)]
            ones_t = ppool.tile([1, 640], BF16, tag="ones")
            scr = ppool.tile([1, 32], F32, tag="scr")
            scrb = ppool.tile([1, 32], BF16, tag="scrb")

            nc.vector.memset(ones_t[:], 1.0)
            nc.vector.memset(scr[:], 0.0)

            # x chunks: all DMAs issued up front (8 bufs). Chunks 0-1 load
            # per k-tile so the first projections can start as soon as the
            # first 128 rows land; later chunks use one bulk DMA each.
            xts = []
            for c in range(NCH):
                xt = xpool.tile([128, NKT, TCH], BF16, tag="xchunk",
                                name=f"x{c}")
                if c < 2:
                    for kt in range(NKT):
                        nc.sync.dma_start(
                            xt[:, kt, :],
                            xT_d[kt * 128:(kt + 1) * 128,
                                 c * TCH:(c + 1) * TCH])
                else:
                    nc.sync.dma_start(
                        xt[:],
                        xT_d[:, c * TCH:(c + 1) * TCH]
                        .rearrange("(k p) t -> p k t", p=128),
                    )
                xts.append(xt)

            # ---- warmup: lift HAM clock gate + preload exp table ----
            # ~3.5us of continuous matmuls so the PE is at K=8/8 before
            # phase A starts; the exp preloads the ACT spline table.
            wu = psG.tile([128, 512], F32, tag="g", name="warm")
            for i in range(36):
                nc.tensor.matmul(wu[:, 0:128], ones_t[0:1, 0:128],
                                 ones_t[0:1, 128:256],
                                 start=(i == 0), stop=(i == 35))
            nc.scalar.activation(scrb[:], wu[0:1, 0:32], AF.Exp)

            # ---- emit helpers ----
            psq = {}

            def phA_qk(c, w_t, b_t, dst, nm, half):
                """Half a projection (4 k-tiles) for a 512-token chunk.

                Split so each bg thunk stays under ~1us of PE time; the two
                halves share one psum accumulator stashed in psq.
                """
                xt = xts[c]
                if half == 0:
                    ps = psq[(nm, c)] = psG.tile([128, 512], F32, tag="g",
                                                 name=f"{nm}{c}")
                else:
                    ps = psq.pop((nm, c))
                for kt in range(4 * half, 4 * half + 4):
                    nc.tensor.matmul(
                        ps[:], w_t[:, kt * 128:(kt + 1) * 128],
                        xt[:, kt, :],
                        start=(kt == 0), stop=(kt == NKT - 1),
                    )
                if half == 1:
                    nc.vector.tensor_scalar_add(
                        dst[:, c * TCH:(c + 1) * TCH], ps[:], b_t[:, 0:1])

            def phA_v(c, half):
                """Natural-layout V for 256 tokens (2 token-tiles)."""
                xt = xts[c]
                for tt in (2 * half, 2 * half + 1):
                    g = c * 4 + tt
                    vps = psG.tile([128, 512], F32, tag="g", name=f"v{g}")
                    for kt in range(NKT):
                        nc.tensor.matmul(
                            vps[:, 0:VW],
                            xt[:, kt, tt * 128:(tt + 1) * 128],
                            wv_t[:, kt, :],
                            start=(kt == 0), stop=False,
                        )
                    nc.tensor.matmul(vps[:, 0:VW], ones_t[0:1, 0:128],
                                     bv_t[:], start=False, stop=True)
                    nc.vector.tensor_copy(Vaug[:, g, :], vps[:, 0:VW])

            def sc_exp(b, qc, kt):
                """Both heads' scoresT + exp for one k-tile; returns ex."""
                q0 = b * L + qc * QC
                ksl = slice(b * L + kt * 128, b * L + (kt + 1) * 128)
                sc = psS.tile([128, 1024], F32, tag="sc")
                nc.tensor.matmul(sc[:, 0:512], KT[0:64, ksl],
                                 QT[0:64, q0:q0 + QC], start=True, stop=True)
                nc.tensor.matmul(sc[:, 512:1024], KT[64:128, ksl],
                                 QT[64:128, q0:q0 + QC], start=True, stop=True)
                ex = epool.tile([128, 1024], BF16, tag="ex")
                nc.scalar.activation(ex[:], sc[:], AF.Exp)
                return ex

            def pv_mm(b, kt, ex, pv0, pv1):
                """PV accumulate for one k-tile (lags sc_exp by 2)."""
                g = b * NKB + kt
                nc.tensor.matmul(pv0[:], Vaug[:, g, 0:DH + 1], ex[:, 0:512],
                                 start=(kt == 0), stop=(kt == NKB - 1))
                nc.tensor.matmul(pv1[:], Vaug[:, g, DH + 1:VW],
                                 ex[:, 512:1024],
                                 start=(kt == 0), stop=(kt == NKB - 1))

            def qc_evac(b, qc, pv0, pv1):
                """Stage pv out of psum fast, then Z reciprocal ->
                broadcast -> normalized attnU (bf16) off the staging copy.

                The two stage copies free the psum accumulators in ~1.5us so
                the next q-chunk's PV can start; the slower recip/broadcast
                chain runs concurrently with the resumed pipeline."""
                # 1/Z: a [1,512] reciprocal costs ~4.3us (iterative divide,
                # one lane), so bounce Z through DRAM to get tokens onto
                # partitions ([128,8] -> 64 cycles), write back, and
                # broadcast-read (stride-0 partition AP) the reciprocal rows
                # into [64,512] multiplier tiles. Z rows are DMA'd straight
                # from psum so the chain starts the moment PV finishes.
                st0 = rzpool.tile([65, 512], F32, tag="st0",
                                  name=f"st0{b}{qc}")
                st1 = rzpool.tile([65, 512], F32, tag="st1",
                                  name=f"st1{b}{qc}")
                nc.vector.tensor_copy(st0[:], pv0[:])
                nc.vector.tensor_copy(st1[:], pv1[:])
                # Serially-dependent DMA legs alternate between the gpsimd
                # and sync queues so each leg head-blocks only its own queue.
                zscr = dpool.tile([2, 512], F32, tag="zscr",
                                  name=f"zs{b}{qc}")
                nc.gpsimd.dma_start(zscr[0:1, :], st0[64:65, :])
                nc.gpsimd.dma_start(zscr[1:2, :], st1[64:65, :])
                zc = zrpool.tile([128, 8], F32, tag="zc", name=f"zc{b}{qc}")
                nc.sync.dma_start(
                    zc[:], zscr[:].rearrange("h (c p) -> p (h c)", p=128))
                zcr = zrpool.tile([128, 8], F32, tag="zcr",
                                  name=f"zcr{b}{qc}")
                nc.vector.reciprocal(zcr[:], zc[:])
                rz2 = dpool.tile([2, 512], F32, tag="rz2", name=f"r2{b}{qc}")
                nc.gpsimd.dma_start(
                    rz2[:].rearrange("h (c p) -> p (h c)", p=128), zcr[:])
                rzm0 = rzpool.tile([64, 512], F32, tag="rzm0",
                                   name=f"rza{b}{qc}")
                rzm1 = rzpool.tile([64, 512], F32, tag="rzm1",
                                   name=f"rzb{b}{qc}")
                r2a, r2b = rz2[0:1, :], rz2[1:2, :]
                nc.sync.dma_start(
                    rzm0[:], AP(r2a.tensor, r2a.offset, [[0, 64], [1, 512]]))
                nc.sync.dma_start(
                    rzm1[:], AP(r2b.tensor, r2b.offset, [[0, 64], [1, 512]]))
                qsl = slice(qc * QC, (qc + 1) * QC)
                nc.vector.scalar_tensor_tensor(
                    attnU[b][0:64, qsl], st0[0:64, :], 1.0, rzm0[:],
                    op0=ALU.mult, op1=ALU.mult)
                nc.vector.scalar_tensor_tensor(
                    attnU[b][64:128, qsl], st1[0:64, :], 1.0, rzm1[:],
                    op0=ALU.mult, op1=ALU.mult)

            def op_unit(b, rc, oc):
                """Output projection for 128 tokens x 512 out-cols."""
                lsl = slice(rc * 128, (rc + 1) * 128)
                rsl = slice(b * L + rc * 128, b * L + (rc + 1) * 128)
                osl = slice(oc * 512, (oc + 1) * 512)
                ps = psG.tile([128, 512], F32, tag="g", name=f"o{b}{rc}{oc}")
                nc.tensor.matmul(ps[:], attnU[b][:, lsl], wo_t[:, osl],
                                 start=True, stop=True)
                ot = opool.tile([128, 512], BF16, tag="ot")
                nc.vector.tensor_copy(ot[:], ps[:])
                nc.sync.dma_start(out_d[rsl, osl], ot[:])

            # ---- software-pipelined emission ----
            # bg FIFO: PE slack work (phase-A halves, out-proj units)
            # consumed one thunk per kt step, placed so each chunk's K/V
            # lands before the kt that needs it and each window stays under
            # the ACT budget. QK halves must occupy ADJACENT slots (they
            # share a rotating psum accumulator).
            bg = []

            def run_bg(n=1):
                for _ in range(min(n, len(bg))):
                    bg.pop(0)()

            def qh(c, h, w_t=None, b_t=None, dst=None, nm=None):
                if w_t is None:
                    w_t, b_t, dst, nm = wq_t, bq_t, QT, "q"
                return lambda: phA_qk(c, w_t, b_t, dst, nm, h)

            def kh(c, h):
                return qh(c, h, wk_t, bk_t, KT, "k")

            def vh(c, h):
                return lambda: phA_v(c, h)

            # fast start: Q/K of chunk 0 inline (unsplit emission order)
            phA_qk(0, wq_t, bq_t, QT, "q", 0)
            phA_qk(0, wq_t, bq_t, QT, "q", 1)
            phA_qk(0, wk_t, bk_t, KT, "k", 0)
            phA_qk(0, wk_t, bk_t, KT, "k", 1)
            def filler():
                """Dep-free matmuls: keep the HAM clock gate open in
                underloaded windows."""
                f = psG.tile([128, 512], F32, tag="g", name="fill")
                for i in range(3):
                    nc.tensor.matmul(f[:], ones_t[0:1, 0:128],
                                     ones_t[0:1, 128:640],
                                     start=(i == 0), stop=(i == 2))

            # per-window background schedules (deadline-ordered; b0-qc0 is
            # structurally oversubscribed, later b0 windows carry one b1
            # chunk each, b1 windows get Q-only plus fillers)
            sched = {
                (0, 0): [vh(0, 0), vh(0, 1), kh(1, 0), kh(1, 1),
                         vh(1, 0), vh(1, 1), kh(2, 0), kh(2, 1),
                         vh(2, 0), vh(2, 1), kh(3, 0), kh(3, 1),
                         vh(3, 0), vh(3, 1), qh(1, 0), qh(1, 1)],
                (0, 1): [qh(2, 0), qh(2, 1), kh(4, 0), kh(4, 1),
                         vh(4, 0), vh(4, 1)],
                (0, 2): [qh(3, 0), qh(3, 1), kh(5, 0), kh(5, 1),
                         vh(5, 0), vh(5, 1)],
                (0, 3): [qh(4, 0), qh(4, 1), kh(6, 0), kh(6, 1),
                         vh(6, 0), vh(6, 1)],
                (1, 0): [qh(5, 0), qh(5, 1), kh(7, 0), kh(7, 1),
                         vh(7, 0), vh(7, 1)],
                (1, 1): [qh(6, 0), qh(6, 1)],
                (1, 2): [qh(7, 0), qh(7, 1)],
            }

            # out-proj units carry their source-window index: a unit's
            # attnU arrives ~8us after its window ends (the Z chain), and
            # the PE runs ~2 exps ahead, so a unit is only eligible two
            # windows later, or in the last quarter of the next window.
            opq = []
            widx = 0

            def pop_op(slot):
                for j, (ws, th) in enumerate(opq):
                    if widx - ws >= 2 or (widx - ws == 1 and slot >= 12):
                        opq.pop(j)
                        return th
                return None

            for b in range(B):
                for qc in range(NQC):
                    bg = sched.get((b, qc), []) + bg
                    pv0 = psPV.tile([65, 512], F32, tag="pv0",
                                    name=f"pv0_{b}{qc}")
                    pv1 = psPV.tile([65, 512], F32, tag="pv1",
                                    name=f"pv1_{b}{qc}")
                    exq = {}
                    for kt in range(NKB):
                        exq[kt] = sc_exp(b, qc, kt)
                        if kt >= 2:
                            pv_mm(b, kt - 2, exq.pop(kt - 2), pv0, pv1)
                        th = None
                        if kt >= 8 or not bg:
                            th = pop_op(kt)
                        if th is not None:
                            th()
                        elif bg:
                            run_bg(1)
                        elif b == 1:
                            filler()
                    pv_mm(b, NKB - 2, exq.pop(NKB - 2), pv0, pv1)
                    pv_mm(b, NKB - 1, exq.pop(NKB - 1), pv0, pv1)
                    qc_evac(b, qc, pv0, pv1)
                    for rc in range(qc * 4, (qc + 1) * 4):
                        for oc in range(2):
                            opq.append((widx, lambda b=b, rc=rc, oc=oc:
                                        op_unit(b, rc, oc)))
                    widx += 1
            # tail: drain remaining background units (fillers keep the
            # clock warm while the last evac chain resolves)
            widx += 1
            while opq:
                opq.pop(0)[1]()
                filler()

    nc.compile()
    _NC_CACHE["nc"] = nc
    return nc


def _shard_inputs(x, W_qkv, b_qkv, W_o):
    import ml_dtypes
    BF = ml_dtypes.bfloat16
    xT = np.ascontiguousarray(
        x.reshape(BL, D_MODEL).T.astype(BF))

    def lhsT_layout(w):
        # [D_MODEL, 128] -> [128, NKT*128] with [p, kt*128+ch] = w[kt*128+p, ch]
        return np.ascontiguousarray(
            w.reshape(NKT, 128, 128).transpose(1, 0, 2)
            .reshape(128, NKT * 128).astype(BF))

    in_maps = []
    for c in range(NCORES):
        cs = slice(c * 128, (c + 1) * 128)
        wq = W_qkv[:, cs] * 0.125
        wk = W_qkv[:, D_MODEL:][:, cs]
        wv = W_qkv[:, 2 * D_MODEL:][:, cs]
        # Wv_aug: [V_h0 | 0 | V_h1 | 0] columns; bias row carries [bv_h0 | 1
        # | bv_h1 | 1] so the ones-row matmul bakes both bias and the softmax
        # ones-columns.
        wv_aug = np.zeros((D_MODEL, VW), dtype=np.float32)
        wv_aug[:, 0:DH] = wv[:, 0:DH]
        wv_aug[:, DH + 1:2 * DH + 1] = wv[:, DH:2 * DH]
        bv = b_qkv[2 * D_MODEL:][cs]
        bv_aug = np.zeros((VW,), dtype=np.float32)
        bv_aug[0:DH] = bv[0:DH]
        bv_aug[DH] = 1.0
        bv_aug[DH + 1:2 * DH + 1] = bv[DH:2 * DH]
        bv_aug[VW - 1] = 1.0
        in_maps.append({
            "xT": xT,
            "wq": lhsT_layout(wq), "wk": lhsT_layout(wk),
            "wv": np.ascontiguousarray(
                wv_aug.reshape(NKT, 128, VW).transpose(1, 0, 2).astype(BF)),
            "bq": np.ascontiguousarray(
                b_qkv[cs] * 0.125, dtype=np.float32).reshape(128, 1),
            "bk": np.ascontiguousarray(
                b_qkv[D_MODEL:][cs], dtype=np.float32).reshape(128, 1),
            "bv": np.ascontiguousarray(bv_aug.astype(BF)).reshape(1, VW),
            "wo": np.ascontiguousarray(W_o[cs, :].astype(BF)),
        })
    return in_maps


def _run(inputs, trace=False, tmpdir=None):
    from concourse.bass_utils import run_bass_kernel_spmd

    _register_ntff_hook()
    nc = _build()
    in_maps = _shard_inputs(
        np.asarray(inputs["x"], dtype=np.float32),
        np.asarray(inputs["W_qkv"], dtype=np.float32),
        np.asarray(inputs["b_qkv"], dtype=np.float32),
        np.asarray(inputs["W_o"], dtype=np.float32),
    )
    res = run_bass_kernel_spmd(nc, in_maps, core_ids=list(range(NCORES)),
                               trace=trace, tmpdir=tmpdir)
    partial = np.zeros((BL, D_MODEL), dtype=np.float32)
    for c in range(NCORES):
        partial += res.results[c]["out"].astype(np.float32)
    out = (partial + np.asarray(inputs["b_o"], dtype=np.float32))
    return out.astype(np.float32).reshape(B, L, D_MODEL), res


def kernel(**inputs) -> np.ndarray:
    out, _ = _run(inputs, trace=False)
    return out


# revision 24
# speedup vs baseline: 1.0147x; 1.0147x over previous
"""Multi-head attention (b=2, l=2048, d_model=1024, h=16) on 8 trn2 NeuronCores.

Sharding: tensor-parallel over heads. Each core owns 2 heads (128 qkv
channels): it computes its QKV projections, attention for its heads, and a
rank-128 partial of the output projection. The host sums the 8 bf16 partials
and adds b_o (the tensor-parallel all-reduce, done at gather time).

v2 design (ACT-paced): all matmul operands bf16 (fp32 psum accumulate).
  phase A: QT/KT [128ch, 4096tok] = W.T @ xT streamed per 512-token chunk;
           V produced directly in natural layout [tok, 130] via xT-stationary
           matmuls against Wv_aug = [V_h0 | 0 | V_h1 | 0] plus a ones-row
           bias matmul that also bakes the softmax ones-columns.
  attention per (b, 512-q-chunk, k-tile): one [128,1024] psum tile holds both
           heads' scoresT (row-group-packed concurrent matmuls); one N=1024
           exp on ACT (the pacer); PV per head accumulates [V_h|1].T @ exp
           into [65, 512] psum over 16 k-tiles (row 64 = softmax denom Z).
  evac per (b, qc): reciprocal(Z) -> gpsimd partition_broadcast -> fused
           multiply: attnU is stored PRE-normalized (bf16), so the output
           projection is a single 128-contraction matmul per [128tok, 512]
           unit, evacuated bf16 and DMA'd out.
  Emission is software-pipelined: phase-A b1 chunks and out-projection units
  are interleaved into the kt-unit stream to fill PE slack under ACT.
"""
import sys
import types

import numpy as np

D_MODEL = 1024
H = 16
DH = 64
B = 2
L = 2048
BL = B * L            # 4096 tokens
NCORES = 8
NKT = D_MODEL // 128  # 8 dmodel tiles
TCH = 512             # phase-A token chunk
NCH = BL // TCH       # 8 chunks
QC = 512              # attention q chunk (per head)
NQC = L // QC         # 4 per batch
NKB = L // 128        # 16 k-tiles per batch
VW = 2 * (DH + 1)     # 130: [V_h0 | 1 | V_h1 | 1]


def _register_ntff_hook():
    """Install the axon NTFF profiling hook module if the image lacks it."""
    if "antenv.axon_hooks" in sys.modules:
        return
    try:
        import antenv
        mod = types.ModuleType("antenv.axon_hooks")
        holder = {}
        mod.set_axon_ntff_profile_hook = lambda h: holder.__setitem__("h", h)
        mod.get_axon_ntff_profile_hook = lambda: holder.get("h")
        sys.modules["antenv.axon_hooks"] = mod
        antenv.axon_hooks = mod
        from trn_agent_boot.trn_boot import _ntff_profile_via_ctypes
        mod.set_axon_ntff_profile_hook(
            _ntff_profile_via_ctypes("/opt/axon/libaxon_pjrt.so")
        )
    except Exception:
        pass


_NC_CACHE = {}


def _build():
    if "nc" in _NC_CACHE:
        return _NC_CACHE["nc"]
    import concourse.bacc as bacc
    import concourse.tile as tile
    import concourse.mybir as mybir
    from concourse.bass import AP

    F32 = mybir.dt.float32
    BF16 = mybir.dt.bfloat16
    AF = mybir.ActivationFunctionType
    ALU = mybir.AluOpType

    nc = bacc.Bacc("TRN2", target_bir_lowering=False, debug=False)

    xT_d = nc.dram_tensor("xT", [D_MODEL, BL], BF16, kind="ExternalInput").ap()
    wq_d = nc.dram_tensor("wq", [128, NKT * 128], BF16, kind="ExternalInput").ap()
    wk_d = nc.dram_tensor("wk", [128, NKT * 128], BF16, kind="ExternalInput").ap()
    wv_d = nc.dram_tensor("wv", [128, NKT, VW], BF16, kind="ExternalInput").ap()
    bq_d = nc.dram_tensor("bq", [128, 1], F32, kind="ExternalInput").ap()
    bk_d = nc.dram_tensor("bk", [128, 1], F32, kind="ExternalInput").ap()
    bv_d = nc.dram_tensor("bv", [1, VW], BF16, kind="ExternalInput").ap()
    wo_d = nc.dram_tensor("wo", [128, D_MODEL], BF16, kind="ExternalInput").ap()
    out_d = nc.dram_tensor("out", [BL, D_MODEL], BF16, kind="ExternalOutput").ap()

    with tile.TileContext(nc) as tc:
        with (
            tc.tile_pool(name="weights", bufs=1) as wpool,
            tc.tile_pool(name="persist", bufs=1) as ppool,
            tc.tile_pool(name="xin", bufs=NCH) as xpool,
            tc.tile_pool(name="expP", bufs=3) as epool,
            tc.tile_pool(name="oout", bufs=3) as opool,
            tc.tile_pool(name="rzP", bufs=2) as rzpool,
            tc.tile_pool(name="zrP", bufs=2) as zrpool,
            tc.tile_pool(name="dram", bufs=2, space="DRAM") as dpool,
            tc.tile_pool(name="psS", bufs=2, space="PSUM") as psS,
            tc.tile_pool(name="psPV", bufs=1, space="PSUM") as psPV,
            tc.tile_pool(name="psG", bufs=2, space="PSUM") as psG,
        ):
            # ---- static tiles ----
            wq_t = wpool.tile([128, NKT * 128], BF16, tag="wq")
            wk_t = wpool.tile([128, NKT * 128], BF16, tag="wk")
            wv_t = wpool.tile([128, NKT, VW], BF16, tag="wv")
            bq_t = wpool.tile([128, 1], F32, tag="bq")
            bk_t = wpool.tile([128, 1], F32, tag="bk")
            bv_t = wpool.tile([1, VW], BF16, tag="bv")
            wo_t = wpool.tile([128, D_MODEL], BF16, tag="wo")
            for t, d in ((wq_t, wq_d), (wk_t, wk_d), (wv_t, wv_d),
                         (bq_t, bq_d), (bk_t, bk_d), (bv_t, bv_d),
                         (wo_t, wo_d)):
                nc.gpsimd.dma_start(t[:], d)

            QT = ppool.tile([128, BL], BF16, tag="QT")
            KT = ppool.tile([128, BL], BF16, tag="KT")
            Vaug = ppool.tile([128, B * NKB, VW], BF16, tag="Vaug")
            attnU = [ppool.tile([128, L], BF16, tag=f"attnU{b}",
                                name=f"attnU{b}") for b in range(B)]
            ones_t = ppool.tile([1, 640], BF16, tag="ones")
            scr = ppool.tile([1, 32], F32, tag="scr")
            scrb = ppool.tile([1, 32], BF16, tag="scrb")

            nc.vector.memset(ones_t[:], 1.0)
            nc.vector.memset(scr[:], 0.0)

            # x chunks: all DMAs issued up front (8 bufs). Chunks 0-1 load
            # per k-tile so the first projections can start as soon as the
            # first 128 rows land; later chunks use one bulk DMA each.
            xts = []
            for c in range(NCH):
                xt = xpool.tile([128, NKT, TCH], BF16, tag="xchunk",
                                name=f"x{c}")
                if c < 2:
                    for kt in range(NKT):
                        nc.sync.dma_start(
                            xt[:, kt, :],
                            xT_d[kt * 128:(kt + 1) * 128,
                                 c * TCH:(c + 1) * TCH])
                else:
                    nc.sync.dma_start(
                        xt[:],
                        xT_d[:, c * TCH:(c + 1) * TCH]
                        .rearrange("(k p) t -> p k t", p=128),
                    )
                xts.append(xt)

            # ---- warmup: lift HAM clock gate + preload exp table ----
            # ~3.5us of continuous matmuls so the PE is at K=8/8 before
            # phase A starts; the exp preloads the ACT spline table.
            wu = psG.tile([128, 512], F32, tag="g", name="warm")
            for i in range(12):
                nc.tensor.matmul(wu[:, 0:128], ones_t[0:1, 0:128],
                                 ones_t[0:1, 128:256],
                                 start=(i == 0), stop=(i == 11))
            nc.scalar.activation(scrb[:], wu[0:1, 0:32], AF.Exp)

            # ---- emit helpers ----
            psq = {}

            def phA_qk(c, w_t, b_t, dst, nm, half):
                """Half a projection (4 k-tiles) for a 512-token chunk.

                Split so each bg thunk stays under ~1us of PE time; the two
                halves share one psum accumulator stashed in psq.
                """
                xt = xts[c]
                if half == 0:
                    ps = psq[(nm, c)] = psG.tile([128, 512], F32, tag="g",
                                                 name=f"{nm}{c}")
                else:
                    ps = psq.pop((nm, c))
                for kt in range(4 * half, 4 * half + 4):
                    nc.tensor.matmul(
                        ps[:], w_t[:, kt * 128:(kt + 1) * 128],
                        xt[:, kt, :],
                        start=(kt == 0), stop=(kt == NKT - 1),
                    )
                if half == 1:
                    nc.vector.tensor_scalar_add(
                        dst[:, c * TCH:(c + 1) * TCH], ps[:], b_t[:, 0:1])

            def phA_v(c, half):
                """Natural-layout V for 256 tokens (2 token-tiles)."""
                xt = xts[c]
                for tt in (2 * half, 2 * half + 1):
                    g = c * 4 + tt
                    vps = psG.tile([128, 512], F32, tag="g", name=f"v{g}")
                    for kt in range(NKT):
                        nc.tensor.matmul(
                            vps[:, 0:VW],
                            xt[:, kt, tt * 128:(tt + 1) * 128],
                            wv_t[:, kt, :],
                            start=(kt == 0), stop=False,
                        )
                    nc.tensor.matmul(vps[:, 0:VW], ones_t[0:1, 0:128],
                                     bv_t[:], start=False, stop=True)
                    nc.vector.tensor_copy(Vaug[:, g, :], vps[:, 0:VW])

            def sc_exp(b, qc, kt):
                """Both heads' scoresT + exp for one k-tile; returns ex."""
                q0 = b * L + qc * QC
                ksl = slice(b * L + kt * 128, b * L + (kt + 1) * 128)
                sc = psS.tile([128, 1024], F32, tag="sc")
                nc.tensor.matmul(sc[:, 0:512], KT[0:64, ksl],
                                 QT[0:64, q0:q0 + QC], start=True, stop=True)
                nc.tensor.matmul(sc[:, 512:1024], KT[64:128, ksl],
                                 QT[64:128, q0:q0 + QC], start=True, stop=True)
                ex = epool.tile([128, 1024], BF16, tag="ex")
                nc.scalar.activation(ex[:], sc[:], AF.Exp)
                return ex

            def pv_mm(b, kt, ex, pv0, pv1):
                """PV accumulate for one k-tile (lags sc_exp by 2)."""
                g = b * NKB + kt
                nc.tensor.matmul(pv0[:], Vaug[:, g, 0:DH + 1], ex[:, 0:512],
                                 start=(kt == 0), stop=(kt == NKB - 1))
                nc.tensor.matmul(pv1[:], Vaug[:, g, DH + 1:VW],
                                 ex[:, 512:1024],
                                 start=(kt == 0), stop=(kt == NKB - 1))

            def qc_evac(b, qc, pv0, pv1):
                """Stage pv out of psum fast, then Z reciprocal ->
                broadcast -> normalized attnU (bf16) off the staging copy.

                The two stage copies free the psum accumulators in ~1.5us so
                the next q-chunk's PV can start; the slower recip/broadcast
                chain runs concurrently with the resumed pipeline."""
                # 1/Z: a [1,512] reciprocal costs ~4.3us (iterative divide,
                # one lane), so bounce Z through DRAM to get tokens onto
                # partitions ([128,8] -> 64 cycles), write back, and
                # broadcast-read (stride-0 partition AP) the reciprocal rows
                # into [64,512] multiplier tiles. Z rows are DMA'd straight
                # from psum so the chain starts the moment PV finishes.
                st0 = rzpool.tile([65, 512], F32, tag="st0",
                                  name=f"st0{b}{qc}")
                st1 = rzpool.tile([65, 512], F32, tag="st1",
                                  name=f"st1{b}{qc}")
                nc.vector.tensor_copy(st0[:], pv0[:])
                nc.vector.tensor_copy(st1[:], pv1[:])
                # Serially-dependent DMA legs alternate between the gpsimd
                # and sync queues so each leg head-blocks only its own queue.
                zscr = dpool.tile([2, 512], F32, tag="zscr",
                                  name=f"zs{b}{qc}")
                nc.sync.dma_start(zscr[0:1, :], st0[64:65, :])
                nc.sync.dma_start(zscr[1:2, :], st1[64:65, :])
                zc = zrpool.tile([128, 8], F32, tag="zc", name=f"zc{b}{qc}")
                nc.sync.dma_start(
                    zc[:], zscr[:].rearrange("h (c p) -> p (h c)", p=128))
                zcr = zrpool.tile([128, 8], F32, tag="zcr",
                                  name=f"zcr{b}{qc}")
                nc.vector.reciprocal(zcr[:], zc[:])
                rz2 = dpool.tile([2, 512], F32, tag="rz2", name=f"r2{b}{qc}")
                nc.sync.dma_start(
                    rz2[:].rearrange("h (c p) -> p (h c)", p=128), zcr[:])
                rzm0 = rzpool.tile([64, 512], F32, tag="rzm0",
                                   name=f"rza{b}{qc}")
                rzm1 = rzpool.tile([64, 512], F32, tag="rzm1",
                                   name=f"rzb{b}{qc}")
                r2a, r2b = rz2[0:1, :], rz2[1:2, :]
                nc.sync.dma_start(
                    rzm0[:], AP(r2a.tensor, r2a.offset, [[0, 64], [1, 512]]))
                nc.sync.dma_start(
                    rzm1[:], AP(r2b.tensor, r2b.offset, [[0, 64], [1, 512]]))
                qsl = slice(qc * QC, (qc + 1) * QC)
                nc.vector.scalar_tensor_tensor(
                    attnU[b][0:64, qsl], st0[0:64, :], 1.0, rzm0[:],
                    op0=ALU.mult, op1=ALU.mult)
                nc.vector.scalar_tensor_tensor(
                    attnU[b][64:128, qsl], st1[0:64, :], 1.0, rzm1[:],
                    op0=ALU.mult, op1=ALU.mult)

            def op_unit(b, rc, oc):
                """Output projection for 128 tokens x 512 out-cols."""
                lsl = slice(rc * 128, (rc + 1) * 128)
                rsl = slice(b * L + rc * 128, b * L + (rc + 1) * 128)
                osl = slice(oc * 512, (oc + 1) * 512)
                ps = psG.tile([128, 512], F32, tag="g", name=f"o{b}{rc}{oc}")
                nc.tensor.matmul(ps[:], attnU[b][:, lsl], wo_t[:, osl],
                                 start=True, stop=True)
                ot = opool.tile([128, 512], BF16, tag="ot")
                nc.vector.tensor_copy(ot[:], ps[:])
                nc.sync.dma_start(out_d[rsl, osl], ot[:])

            # ---- software-pipelined emission ----
            # bg FIFO: PE slack work (phase-A halves, out-proj units)
            # consumed one thunk per kt step, placed so each chunk's K/V
            # lands before the kt that needs it and each window stays under
            # the ACT budget. QK halves must occupy ADJACENT slots (they
            # share a rotating psum accumulator).
            bg = []

            def run_bg(n=1):
                for _ in range(min(n, len(bg))):
                    bg.pop(0)()

            def qh(c, h, w_t=None, b_t=None, dst=None, nm=None):
                if w_t is None:
                    w_t, b_t, dst, nm = wq_t, bq_t, QT, "q"
                return lambda: phA_qk(c, w_t, b_t, dst, nm, h)

            def kh(c, h):
                return qh(c, h, wk_t, bk_t, KT, "k")

            def vh(c, h):
                return lambda: phA_v(c, h)

            # fast start: Q/K of chunk 0 inline (unsplit emission order)
            phA_qk(0, wq_t, bq_t, QT, "q", 0)
            phA_qk(0, wq_t, bq_t, QT, "q", 1)
            phA_qk(0, wk_t, bk_t, KT, "k", 0)
            phA_qk(0, wk_t, bk_t, KT, "k", 1)
            def filler():
                """Dep-free matmuls: keep the HAM clock gate open in
                underloaded windows."""
                f = psG.tile([128, 512], F32, tag="g", name="fill")
                for i in range(3):
                    nc.tensor.matmul(f[:], ones_t[0:1, 0:128],
                                     ones_t[0:1, 128:640],
                                     start=(i == 0), stop=(i == 2))

            # per-window background schedules (deadline-ordered; b0-qc0 is
            # structurally oversubscribed, later b0 windows carry one b1
            # chunk each, b1 windows get Q-only plus fillers)
            sched = {
                (0, 0): [vh(0, 0), vh(0, 1), kh(1, 0), kh(1, 1),
                         vh(1, 0), vh(1, 1), kh(2, 0), kh(2, 1),
                         vh(2, 0), vh(2, 1), kh(3, 0), kh(3, 1),
                         vh(3, 0), vh(3, 1), qh(1, 0), qh(1, 1)],
                (0, 1): [qh(2, 0), qh(2, 1), kh(4, 0), kh(4, 1),
                         vh(4, 0), vh(4, 1)],
                (0, 2): [qh(3, 0), qh(3, 1), kh(5, 0), kh(5, 1),
                         vh(5, 0), vh(5, 1)],
                (0, 3): [qh(4, 0), qh(4, 1), kh(6, 0), kh(6, 1),
                         vh(6, 0), vh(6, 1)],
                (1, 0): [qh(5, 0), qh(5, 1), kh(7, 0), kh(7, 1),
                         vh(7, 0), vh(7, 1)],
                (1, 1): [qh(6, 0), qh(6, 1)],
                (1, 2): [qh(7, 0), qh(7, 1)],
            }

            # out-proj units carry their source-window index: a unit's
            # attnU arrives ~8us after its window ends (the Z chain), and
            # the PE runs ~2 exps ahead, so a unit is only eligible two
            # windows later, or in the last quarter of the next window.
            opq = []
            widx = 0

            def pop_op(slot):
                for j, (ws, th) in enumerate(opq):
                    if widx - ws >= 2:
                        opq.pop(j)
                        return th
                return None

            for b in range(B):
                for qc in range(NQC):
                    bg = sched.get((b, qc), []) + bg
                    pv0 = psPV.tile([65, 512], F32, tag="pv0",
                                    name=f"pv0_{b}{qc}")
                    pv1 = psPV.tile([65, 512], F32, tag="pv1",
                                    name=f"pv1_{b}{qc}")
                    exq = {}
                    for kt in range(NKB):
                        exq[kt] = sc_exp(b, qc, kt)
                        if kt >= 2:
                            pv_mm(b, kt - 2, exq.pop(kt - 2), pv0, pv1)
                        th = None
                        if kt >= 8 or not bg:
                            th = pop_op(kt)
                        if th is not None:
                            th()
                        elif bg:
                            run_bg(1)
                        elif b == 1:
                            filler()
                    pv_mm(b, NKB - 2, exq.pop(NKB - 2), pv0, pv1)
                    pv_mm(b, NKB - 1, exq.pop(NKB - 1), pv0, pv1)
                    qc_evac(b, qc, pv0, pv1)
                    for rc in range(qc * 4, (qc + 1) * 4):
                        for oc in range(2):
                            opq.append((widx, lambda b=b, rc=rc, oc=oc:
                                        op_unit(b, rc, oc)))
                    widx += 1
            # tail: drain remaining background units (fillers keep the
            # clock warm while the last evac chain resolves)
            widx += 1
            while opq:
                opq.pop(0)[1]()
                filler()

    nc.compile()
    _NC_CACHE["nc"] = nc
    return nc


def _shard_inputs(x, W_qkv, b_qkv, W_o):
    import ml_dtypes
    BF = ml_dtypes.bfloat16
    xT = np.ascontiguousarray(
        x.reshape(BL, D_MODEL).T.astype(BF))

    def lhsT_layout(w):
        # [D_MODEL, 128] -> [128, NKT*128] with [p, kt*128+ch] = w[kt*128+p, ch]
        return np.ascontiguousarray(
            w.reshape(NKT, 128, 128).transpose(1, 0, 2)
            .reshape(128, NKT * 128).astype(BF))

    in_maps = []
    for c in range(NCORES):
        cs = slice(c * 128, (c + 1) * 128)
        wq = W_qkv[:, cs] * 0.125
        wk = W_qkv[:, D_MODEL:][:, cs]
        wv = W_qkv[:, 2 * D_MODEL:][:, cs]
        # Wv_aug: [V_h0 | 0 | V_h1 | 0] columns; bias row carries [bv_h0 | 1
        # | bv_h1 | 1] so the ones-row matmul bakes both bias and the softmax
        # ones-columns.
        wv_aug = np.zeros((D_MODEL, VW), dtype=np.float32)
        wv_aug[:, 0:DH] = wv[:, 0:DH]
        wv_aug[:, DH + 1:2 * DH + 1] = wv[:, DH:2 * DH]
        bv = b_qkv[2 * D_MODEL:][cs]
        bv_aug = np.zeros((VW,), dtype=np.float32)
        bv_aug[0:DH] = bv[0:DH]
        bv_aug[DH] = 1.0
        bv_aug[DH + 1:2 * DH + 1] = bv[DH:2 * DH]
        bv_aug[VW - 1] = 1.0
        in_maps.append({
            "xT": xT,
            "wq": lhsT_layout(wq), "wk": lhsT_layout(wk),
            "wv": np.ascontiguousarray(
                wv_aug.reshape(NKT, 128, VW).transpose(1, 0, 2).astype(BF)),
            "bq": np.ascontiguousarray(
                b_qkv[cs] * 0.125, dtype=np.float32).reshape(128, 1),
            "bk": np.ascontiguousarray(
                b_qkv[D_MODEL:][cs], dtype=np.float32).reshape(128, 1),
            "bv": np.ascontiguousarray(bv_aug.astype(BF)).reshape(1, VW),
            "wo": np.ascontiguousarray(W_o[cs, :].astype(BF)),
        })
    return in_maps


def _run(inputs, trace=False, tmpdir=None):
    from concourse.bass_utils import run_bass_kernel_spmd

    _register_ntff_hook()
    nc = _build()
    in_maps = _shard_inputs(
        np.asarray(inputs["x"], dtype=np.float32),
        np.asarray(inputs["W_qkv"], dtype=np.float32),
        np.asarray(inputs["b_qkv"], dtype=np.float32),
        np.asarray(inputs["W_o"], dtype=np.float32),
    )
    res = run_bass_kernel_spmd(nc, in_maps, core_ids=list(range(NCORES)),
                               trace=trace, tmpdir=tmpdir)
    partial = np.zeros((BL, D_MODEL), dtype=np.float32)
    for c in range(NCORES):
        partial += res.results[c]["out"].astype(np.float32)
    out = (partial + np.asarray(inputs["b_o"], dtype=np.float32))
    return out.astype(np.float32).reshape(B, L, D_MODEL), res


def kernel(**inputs) -> np.ndarray:
    out, _ = _run(inputs, trace=False)
    return out


# revision 25
# speedup vs baseline: 1.2593x; 1.2411x over previous
"""Multi-head attention (b=2, l=2048, d_model=1024, h=16) on 8 trn2 NeuronCores.

Sharding: tensor-parallel over heads. Each core owns 2 heads (128 qkv
channels): it computes its QKV projections, attention for its heads, and a
rank-128 partial of the output projection. The host sums the 8 bf16 partials
and adds b_o (the tensor-parallel all-reduce, done at gather time).

v2 design (ACT-paced): all matmul operands bf16 (fp32 psum accumulate).
  phase A: QT/KT [128ch, 4096tok] = W.T @ xT streamed per 512-token chunk;
           V produced directly in natural layout [tok, 130] via xT-stationary
           matmuls against Wv_aug = [V_h0 | 0 | V_h1 | 0] plus a ones-row
           bias matmul that also bakes the softmax ones-columns.
  attention per (b, 512-q-chunk, k-tile): one [128,1024] psum tile holds both
           heads' scoresT (row-group-packed concurrent matmuls); one N=1024
           exp on ACT (the pacer); PV per head accumulates [V_h|1].T @ exp
           into [65, 512] psum over 16 k-tiles (row 64 = softmax denom Z).
  evac per (b, qc): reciprocal(Z) -> gpsimd partition_broadcast -> fused
           multiply: attnU is stored PRE-normalized (bf16), so the output
           projection is a single 128-contraction matmul per [128tok, 512]
           unit, evacuated bf16 and DMA'd out.
  Emission is software-pipelined: phase-A b1 chunks and out-projection units
  are interleaved into the kt-unit stream to fill PE slack under ACT.
"""
import sys
import types

import numpy as np

D_MODEL = 1024
H = 16
DH = 64
B = 2
L = 2048
BL = B * L            # 4096 tokens
NCORES = 8
NKT = D_MODEL // 128  # 8 dmodel tiles
TCH = 512             # phase-A token chunk
NCH = BL // TCH       # 8 chunks
QC = 512              # attention q chunk (per head)
NQC = L // QC         # 4 per batch
NKB = L // 128        # 16 k-tiles per batch
VW = 2 * (DH + 1)     # 130: [V_h0 | 1 | V_h1 | 1]


def _register_ntff_hook():
    """Install the axon NTFF profiling hook module if the image lacks it."""
    if "antenv.axon_hooks" in sys.modules:
        return
    try:
        import antenv
        mod = types.ModuleType("antenv.axon_hooks")
        holder = {}
        mod.set_axon_ntff_profile_hook = lambda h: holder.__setitem__("h", h)
        mod.get_axon_ntff_profile_hook = lambda: holder.get("h")
        sys.modules["antenv.axon_hooks"] = mod
        antenv.axon_hooks = mod
        from trn_agent_boot.trn_boot import _ntff_profile_via_ctypes
        mod.set_axon_ntff_profile_hook(
            _ntff_profile_via_ctypes("/opt/axon/libaxon_pjrt.so")
        )
    except Exception:
        pass


_NC_CACHE = {}


def _build():
    if "nc" in _NC_CACHE:
        return _NC_CACHE["nc"]
    import concourse.bacc as bacc
    import concourse.tile as tile
    import concourse.mybir as mybir
    from concourse.bass import AP

    F32 = mybir.dt.float32
    BF16 = mybir.dt.bfloat16
    AF = mybir.ActivationFunctionType
    ALU = mybir.AluOpType

    nc = bacc.Bacc("TRN2", target_bir_lowering=False, debug=False)

    xT_d = nc.dram_tensor("xT", [D_MODEL, BL], BF16, kind="ExternalInput").ap()
    wq_d = nc.dram_tensor("wq", [128, NKT * 128], BF16, kind="ExternalInput").ap()
    wk_d = nc.dram_tensor("wk", [128, NKT * 128], BF16, kind="ExternalInput").ap()
    wv_d = nc.dram_tensor("wv", [128, NKT, VW], BF16, kind="ExternalInput").ap()
    bq_d = nc.dram_tensor("bq", [128, 1], F32, kind="ExternalInput").ap()
    bk_d = nc.dram_tensor("bk", [128, 1], F32, kind="ExternalInput").ap()
    bv_d = nc.dram_tensor("bv", [1, VW], BF16, kind="ExternalInput").ap()
    wo_d = nc.dram_tensor("wo", [128, D_MODEL], BF16, kind="ExternalInput").ap()
    out_d = nc.dram_tensor("out", [BL, D_MODEL], BF16, kind="ExternalOutput").ap()

    with tile.TileContext(nc) as tc:
        with (
            tc.tile_pool(name="weights", bufs=1) as wpool,
            tc.tile_pool(name="persist", bufs=1) as ppool,
            tc.tile_pool(name="xin", bufs=NCH) as xpool,
            tc.tile_pool(name="expP", bufs=3) as epool,
            tc.tile_pool(name="oout", bufs=3) as opool,
            tc.tile_pool(name="rzP", bufs=2) as rzpool,
            tc.tile_pool(name="zrP", bufs=2) as zrpool,
            tc.tile_pool(name="dram", bufs=2, space="DRAM") as dpool,
            tc.tile_pool(name="psS", bufs=2, space="PSUM") as psS,
            tc.tile_pool(name="psPV", bufs=1, space="PSUM") as psPV,
            tc.tile_pool(name="psG", bufs=2, space="PSUM") as psG,
        ):
            # ---- static tiles ----
            wq_t = wpool.tile([128, NKT * 128], BF16, tag="wq")
            wk_t = wpool.tile([128, NKT * 128], BF16, tag="wk")
            wv_t = wpool.tile([128, NKT, VW], BF16, tag="wv")
            bq_t = wpool.tile([128, 1], F32, tag="bq")
            bk_t = wpool.tile([128, 1], F32, tag="bk")
            bv_t = wpool.tile([1, VW], BF16, tag="bv")
            wo_t = wpool.tile([128, D_MODEL], BF16, tag="wo")
            for t, d in ((wq_t, wq_d), (wk_t, wk_d), (wv_t, wv_d),
                         (bq_t, bq_d), (bk_t, bk_d), (bv_t, bv_d),
                         (wo_t, wo_d)):
                nc.gpsimd.dma_start(t[:], d)

            QT = ppool.tile([128, BL], BF16, tag="QT")
            KT = ppool.tile([128, BL], BF16, tag="KT")
            Vaug = ppool.tile([128, B * NKB, VW], BF16, tag="Vaug")
            attnU = [ppool.tile([128, L], BF16, tag=f"attnU{b}",
                                name=f"attnU{b}") for b in range(B)]
            ones_t = ppool.tile([1, 640], BF16, tag="ones")
            scr = ppool.tile([1, 32], F32, tag="scr")
            scrb = ppool.tile([1, 32], BF16, tag="scrb")

            nc.vector.memset(ones_t[:], 1.0)
            nc.vector.memset(scr[:], 0.0)

            # x chunks: all DMAs issued up front (8 bufs). Chunks 0-1 load
            # per k-tile so the first projections can start as soon as the
            # first 128 rows land; later chunks use one bulk DMA each.
            xts = []
            for c in range(NCH):
                xt = xpool.tile([128, NKT, TCH], BF16, tag="xchunk",
                                name=f"x{c}")
                if c < 2:
                    for kt in range(NKT):
                        nc.sync.dma_start(
                            xt[:, kt, :],
                            xT_d[kt * 128:(kt + 1) * 128,
                                 c * TCH:(c + 1) * TCH])
                else:
                    nc.sync.dma_start(
                        xt[:],
                        xT_d[:, c * TCH:(c + 1) * TCH]
                        .rearrange("(k p) t -> p k t", p=128),
                    )
                xts.append(xt)

            # ---- warmup: lift HAM clock gate + preload exp table ----
            # ~3.5us of continuous matmuls so the PE is at K=8/8 before
            # phase A starts; the exp preloads the ACT spline table.
            wu = psG.tile([128, 512], F32, tag="g", name="warm")
            for i in range(36):
                nc.tensor.matmul(wu[:, 0:128], ones_t[0:1, 0:128],
                                 ones_t[0:1, 128:256],
                                 start=(i == 0), stop=(i == 35))
            nc.scalar.activation(scrb[:], wu[0:1, 0:32], AF.Exp)

            # ---- emit helpers ----
            psq = {}

            def phA_qk(c, w_t, b_t, dst, nm, half):
                """Half a projection (4 k-tiles) for a 512-token chunk.

                Split so each bg thunk stays under ~1us of PE time; the two
                halves share one psum accumulator stashed in psq.
                """
                xt = xts[c]
                if half == 0:
                    ps = psq[(nm, c)] = psG.tile([128, 512], F32, tag="g",
                                                 name=f"{nm}{c}")
                else:
                    ps = psq.pop((nm, c))
                for kt in range(4 * half, 4 * half + 4):
                    nc.tensor.matmul(
                        ps[:], w_t[:, kt * 128:(kt + 1) * 128],
                        xt[:, kt, :],
                        start=(kt == 0), stop=(kt == NKT - 1),
                    )
                if half == 1:
                    nc.vector.tensor_scalar_add(
                        dst[:, c * TCH:(c + 1) * TCH], ps[:], b_t[:, 0:1])

            def phA_v(c, half):
                """Natural-layout V for 256 tokens (2 token-tiles)."""
                xt = xts[c]
                for tt in (2 * half, 2 * half + 1):
                    g = c * 4 + tt
                    vps = psG.tile([128, 512], F32, tag="g", name=f"v{g}")
                    for kt in range(NKT):
                        nc.tensor.matmul(
                            vps[:, 0:VW],
                            xt[:, kt, tt * 128:(tt + 1) * 128],
                            wv_t[:, kt, :],
                            start=(kt == 0), stop=False,
                        )
                    nc.tensor.matmul(vps[:, 0:VW], ones_t[0:1, 0:128],
                                     bv_t[:], start=False, stop=True)
                    nc.vector.tensor_copy(Vaug[:, g, :], vps[:, 0:VW])

            def sc_exp(b, qc, kt):
                """Both heads' scoresT + exp for one k-tile; returns ex."""
                q0 = b * L + qc * QC
                ksl = slice(b * L + kt * 128, b * L + (kt + 1) * 128)
                sc = psS.tile([128, 1024], F32, tag="sc")
                nc.tensor.matmul(sc[:, 0:512], KT[0:64, ksl],
                                 QT[0:64, q0:q0 + QC], start=True, stop=True)
                nc.tensor.matmul(sc[:, 512:1024], KT[64:128, ksl],
                                 QT[64:128, q0:q0 + QC], start=True, stop=True)
                ex = epool.tile([128, 1024], BF16, tag="ex")
                nc.scalar.activation(ex[:], sc[:], AF.Exp)
                return ex

            def pv_mm(b, kt, ex, pv0, pv1):
                """PV accumulate for one k-tile (lags sc_exp by 2)."""
                g = b * NKB + kt
                nc.tensor.matmul(pv0[:], Vaug[:, g, 0:DH + 1], ex[:, 0:512],
                                 start=(kt == 0), stop=(kt == NKB - 1))
                nc.tensor.matmul(pv1[:], Vaug[:, g, DH + 1:VW],
                                 ex[:, 512:1024],
                                 start=(kt == 0), stop=(kt == NKB - 1))

            def qc_evac(b, qc, pv0, pv1):
                """Stage pv out of psum fast, then Z reciprocal ->
                broadcast -> normalized attnU (bf16) off the staging copy.

                The two stage copies free the psum accumulators in ~1.5us so
                the next q-chunk's PV can start; the slower recip/broadcast
                chain runs concurrently with the resumed pipeline."""
                # 1/Z: a [1,512] reciprocal costs ~4.3us (iterative divide,
                # one lane), so bounce Z through DRAM to get tokens onto
                # partitions ([128,8] -> 64 cycles), write back, and
                # broadcast-read (stride-0 partition AP) the reciprocal rows
                # into [64,512] multiplier tiles. Z rows are DMA'd straight
                # from psum so the chain starts the moment PV finishes.
                st0 = rzpool.tile([65, 512], F32, tag="st0",
                                  name=f"st0{b}{qc}")
                st1 = rzpool.tile([65, 512], F32, tag="st1",
                                  name=f"st1{b}{qc}")
                nc.vector.tensor_copy(st0[:], pv0[:])
                nc.vector.tensor_copy(st1[:], pv1[:])
                # Serially-dependent DMA legs alternate between the gpsimd
                # and sync queues so each leg head-blocks only its own queue.
                zscr = dpool.tile([2, 512], F32, tag="zscr",
                                  name=f"zs{b}{qc}")
                nc.sync.dma_start(zscr[0:1, :], st0[64:65, :])
                nc.sync.dma_start(zscr[1:2, :], st1[64:65, :])
                zc = zrpool.tile([128, 8], F32, tag="zc", name=f"zc{b}{qc}")
                nc.sync.dma_start(
                    zc[:], zscr[:].rearrange("h (c p) -> p (h c)", p=128))
                zcr = zrpool.tile([128, 8], F32, tag="zcr",
                                  name=f"zcr{b}{qc}")
                nc.vector.reciprocal(zcr[:], zc[:])
                rz2 = dpool.tile([2, 512], F32, tag="rz2", name=f"r2{b}{qc}")
                nc.sync.dma_start(
                    rz2[:].rearrange("h (c p) -> p (h c)", p=128), zcr[:])
                rzm0 = rzpool.tile([64, 512], F32, tag="rzm0",
                                   name=f"rza{b}{qc}")
                rzm1 = rzpool.tile([64, 512], F32, tag="rzm1",
                                   name=f"rzb{b}{qc}")
                r2a, r2b = rz2[0:1, :], rz2[1:2, :]
                nc.sync.dma_start(
                    rzm0[:], AP(r2a.tensor, r2a.offset, [[0, 64], [1, 512]]))
                nc.sync.dma_start(
                    rzm1[:], AP(r2b.tensor, r2b.offset, [[0, 64], [1, 512]]))
                qsl = slice(qc * QC, (qc + 1) * QC)
                nc.vector.scalar_tensor_tensor(
                    attnU[b][0:64, qsl], st0[0:64, :], 1.0, rzm0[:],
                    op0=ALU.mult, op1=ALU.mult)
                nc.vector.scalar_tensor_tensor(
                    attnU[b][64:128, qsl], st1[0:64, :], 1.0, rzm1[:],
                    op0=ALU.mult, op1=ALU.mult)

            def op_unit(b, rc, oc):
                """Output projection for 128 tokens x 512 out-cols."""
                lsl = slice(rc * 128, (rc + 1) * 128)
                rsl = slice(b * L + rc * 128, b * L + (rc + 1) * 128)
                osl = slice(oc * 512, (oc + 1) * 512)
                ps = psG.tile([128, 512], F32, tag="g", name=f"o{b}{rc}{oc}")
                nc.tensor.matmul(ps[:], attnU[b][:, lsl], wo_t[:, osl],
                                 start=True, stop=True)
                ot = opool.tile([128, 512], BF16, tag="ot")
                nc.vector.tensor_copy(ot[:], ps[:])
                nc.sync.dma_start(out_d[rsl, osl], ot[:])

            # ---- software-pipelined emission ----
            # bg FIFO: PE slack work (phase-A halves, out-proj units)
            # consumed one thunk per kt step, placed so each chunk's K/V
            # lands before the kt that needs it and each window stays under
            # the ACT budget. QK halves must occupy ADJACENT slots (they
            # share a rotating psum accumulator).
            bg = []

            def run_bg(n=1):
                for _ in range(min(n, len(bg))):
                    bg.pop(0)()

            def qh(c, h, w_t=None, b_t=None, dst=None, nm=None):
                if w_t is None:
                    w_t, b_t, dst, nm = wq_t, bq_t, QT, "q"
                return lambda: phA_qk(c, w_t, b_t, dst, nm, h)

            def kh(c, h):
                return qh(c, h, wk_t, bk_t, KT, "k")

            def vh(c, h):
                return lambda: phA_v(c, h)

            # fast start: Q/K of chunk 0 inline (unsplit emission order)
            phA_qk(0, wq_t, bq_t, QT, "q", 0)
            phA_qk(0, wq_t, bq_t, QT, "q", 1)
            phA_qk(0, wk_t, bk_t, KT, "k", 0)
            phA_qk(0, wk_t, bk_t, KT, "k", 1)
            def filler():
                """Dep-free matmuls: keep the HAM clock gate open in
                underloaded windows."""
                f = psG.tile([128, 512], F32, tag="g", name="fill")
                for i in range(3):
                    nc.tensor.matmul(f[:], ones_t[0:1, 0:128],
                                     ones_t[0:1, 128:640],
                                     start=(i == 0), stop=(i == 2))

            # per-window background schedules (deadline-ordered; b0-qc0 is
            # structurally oversubscribed, later b0 windows carry one b1
            # chunk each, b1 windows get Q-only plus fillers)
            sched = {
                (0, 0): [vh(0, 0), vh(0, 1), kh(1, 0), kh(1, 1),
                         vh(1, 0), vh(1, 1), kh(2, 0), kh(2, 1),
                         vh(2, 0), vh(2, 1), kh(3, 0), kh(3, 1),
                         vh(3, 0), vh(3, 1), qh(1, 0), qh(1, 1)],
                (0, 1): [qh(2, 0), qh(2, 1), kh(4, 0), kh(4, 1),
                         vh(4, 0), vh(4, 1)],
                (0, 2): [qh(3, 0), qh(3, 1), kh(5, 0), kh(5, 1),
                         vh(5, 0), vh(5, 1)],
                (0, 3): [qh(4, 0), qh(4, 1), kh(6, 0), kh(6, 1),
                         vh(6, 0), vh(6, 1)],
                (1, 0): [qh(5, 0), qh(5, 1), kh(7, 0), kh(7, 1),
                         vh(7, 0), vh(7, 1)],
                (1, 1): [qh(6, 0), qh(6, 1)],
                (1, 2): [qh(7, 0), qh(7, 1)],
            }

            # out-proj units carry their source-window index: a unit's
            # attnU arrives ~8us after its window ends (the Z chain), and
            # the PE runs ~2 exps ahead, so a unit is only eligible two
            # windows later, or in the last quarter of the next window.
            opq = []
            widx = 0

            def pop_op(slot):
                for j, (ws, th) in enumerate(opq):
                    if widx - ws >= 2:
                        opq.pop(j)
                        return th
                return None

            for b in range(B):
                for qc in range(NQC):
                    bg = sched.get((b, qc), []) + bg
                    pv0 = psPV.tile([65, 512], F32, tag="pv0",
                                    name=f"pv0_{b}{qc}")
                    pv1 = psPV.tile([65, 512], F32, tag="pv1",
                                    name=f"pv1_{b}{qc}")
                    exq = {}
                    for kt in range(NKB):
                        exq[kt] = sc_exp(b, qc, kt)
                        if kt >= 2:
                            pv_mm(b, kt - 2, exq.pop(kt - 2), pv0, pv1)
                        th = None
                        if kt >= 8 or not bg:
                            th = pop_op(kt)
                        if th is not None:
                            th()
                        elif bg:
                            run_bg(1)
                    pv_mm(b, NKB - 2, exq.pop(NKB - 2), pv0, pv1)
                    pv_mm(b, NKB - 1, exq.pop(NKB - 1), pv0, pv1)
                    qc_evac(b, qc, pv0, pv1)
                    for rc in range(qc * 4, (qc + 1) * 4):
                        for oc in range(2):
                            opq.append((widx, lambda b=b, rc=rc, oc=oc:
                                        op_unit(b, rc, oc)))
                    widx += 1
            # tail: drain remaining background units (fillers keep the
            # clock warm while the last evac chain resolves)
            widx += 1
            while opq:
                opq.pop(0)[1]()

    nc.compile()
    _NC_CACHE["nc"] = nc
    return nc


def _shard_inputs(x, W_qkv, b_qkv, W_o):
    import ml_dtypes
    BF = ml_dtypes.bfloat16
    xT = np.ascontiguousarray(
        x.reshape(BL, D_MODEL).T.astype(BF))

    def lhsT_layout(w):
        # [D_MODEL, 128] -> [128, NKT*128] with [p, kt*128+ch] = w[kt*128+p, ch]
        return np.ascontiguousarray(
            w.reshape(NKT, 128, 128).transpose(1, 0, 2)
            .reshape(128, NKT * 128).astype(BF))

    in_maps = []
    for c in range(NCORES):
        cs = slice(c * 128, (c + 1) * 128)
        wq = W_qkv[:, cs] * 0.125
        wk = W_qkv[:, D_MODEL:][:, cs]
        wv = W_qkv[:, 2 * D_MODEL:][:, cs]
        # Wv_aug: [V_h0 | 0 | V_h1 | 0] columns; bias row carries [bv_h0 | 1
        # | bv_h1 | 1] so the ones-row matmul bakes both bias and the softmax
        # ones-columns.
        wv_aug = np.zeros((D_MODEL, VW), dtype=np.float32)
        wv_aug[:, 0:DH] = wv[:, 0:DH]
        wv_aug[:, DH + 1:2 * DH + 1] = wv[:, DH:2 * DH]
        bv = b_qkv[2 * D_MODEL:][cs]
        bv_aug = np.zeros((VW,), dtype=np.float32)
        bv_aug[0:DH] = bv[0:DH]
        bv_aug[DH] = 1.0
        bv_aug[DH + 1:2 * DH + 1] = bv[DH:2 * DH]
        bv_aug[VW - 1] = 1.0
        in_maps.append({
            "xT": xT,
            "wq": lhsT_layout(wq), "wk": lhsT_layout(wk),
            "wv": np.ascontiguousarray(
                wv_aug.reshape(NKT, 128, VW).transpose(1, 0, 2).astype(BF)),
            "bq": np.ascontiguousarray(
                b_qkv[cs] * 0.125, dtype=np.float32).reshape(128, 1),
            "bk": np.ascontiguousarray(
                b_qkv[D_MODEL:][cs], dtype=np.float32).reshape(128, 1),
            "bv": np.ascontiguousarray(bv_aug.astype(BF)).reshape(1, VW),
            "wo": np.ascontiguousarray(W_o[cs, :].astype(BF)),
        })
    return in_maps


def _run(inputs, trace=False, tmpdir=None):
    from concourse.bass_utils import run_bass_kernel_spmd

    _register_ntff_hook()
    nc = _build()
    in_maps = _shard_inputs(
        np.asarray(inputs["x"], dtype=np.float32),
        np.asarray(inputs["W_qkv"], dtype=np.float32),
        np.asarray(inputs["b_qkv"], dtype=np.float32),
        np.asarray(inputs["W_o"], dtype=np.float32),
    )
    res = run_bass_kernel_spmd(nc, in_maps, core_ids=list(range(NCORES)),
                               trace=trace, tmpdir=tmpdir)
    partial = np.zeros((BL, D_MODEL), dtype=np.float32)
    for c in range(NCORES):
        partial += res.results[c]["out"].astype(np.float32)
    out = (partial + np.asarray(inputs["b_o"], dtype=np.float32))
    return out.astype(np.float32).reshape(B, L, D_MODEL), res


def kernel(**inputs) -> np.ndarray:
    out, _ = _run(inputs, trace=False)
    return out


# revision 36
# speedup vs baseline: 1.7088x; 1.3569x over previous
"""Multi-head attention (b=2, l=2048, d_model=1024, h=16) on 8 trn2 NeuronCores.

Sharding: tensor-parallel over heads. Each core owns 2 heads (128 qkv
channels): it computes its QKV projections, attention for its heads, and a
rank-128 partial of the output projection. The host sums the 8 bf16 partials
and adds b_o (the tensor-parallel all-reduce, done at gather time).

v2 design (ACT-paced): all matmul operands bf16 (fp32 psum accumulate).
  phase A: QT/KT [128ch, 4096tok] = W.T @ xT streamed per 512-token chunk;
           V produced directly in natural layout [tok, 130] via xT-stationary
           matmuls against Wv_aug = [V_h0 | 0 | V_h1 | 0] plus a ones-row
           bias matmul that also bakes the softmax ones-columns.
  attention per (b, 512-q-chunk, k-tile): one [128,1024] psum tile holds both
           heads' scoresT (row-group-packed concurrent matmuls); one N=1024
           exp on ACT (the pacer); PV per head accumulates [V_h|1].T @ exp
           into [65, 512] psum over 16 k-tiles (row 64 = softmax denom Z).
  evac per (b, qc): Z row -> DVE StreamTranspose -> strided reciprocal
           ([32,16] view, 16 elems/lane) -> transpose back -> gpsimd
           partition_broadcast -> fused multiply. attnU is stored
           PRE-normalized (bf16), so the output projection is a single
           128-contraction matmul per [128tok, 512] unit, evacuated bf16
           and DMA'd out. The chain is engine-internal (zero DMA legs) so
           the in-order DVE/sync queues never head-block on it.
  Emission is software-pipelined: phase-A b1 chunks and out-projection
  units are interleaved into the kt-unit stream to fill PE slack under the
  ACT-paced exp stream; out-proj units are age-gated one window behind
  their qc so their attnU dependency is always resolved.
"""
import sys
import types

import numpy as np

D_MODEL = 1024
H = 16
DH = 64
B = 2
L = 2048
BL = B * L            # 4096 tokens
NCORES = 8
NKT = D_MODEL // 128  # 8 dmodel tiles
TCH = 512             # phase-A token chunk
NCH = BL // TCH       # 8 chunks
QC = 512              # attention q chunk (per head)
NQC = L // QC         # 4 per batch
NKB = L // 128        # 16 k-tiles per batch
VW = 2 * (DH + 1)     # 130: [V_h0 | 1 | V_h1 | 1]


def _register_ntff_hook():
    """Install the axon NTFF profiling hook module if the image lacks it."""
    if "antenv.axon_hooks" in sys.modules:
        return
    try:
        import antenv
        mod = types.ModuleType("antenv.axon_hooks")
        holder = {}
        mod.set_axon_ntff_profile_hook = lambda h: holder.__setitem__("h", h)
        mod.get_axon_ntff_profile_hook = lambda: holder.get("h")
        sys.modules["antenv.axon_hooks"] = mod
        antenv.axon_hooks = mod
        from trn_agent_boot.trn_boot import _ntff_profile_via_ctypes
        mod.set_axon_ntff_profile_hook(
            _ntff_profile_via_ctypes("/opt/axon/libaxon_pjrt.so")
        )
    except Exception:
        pass


_NC_CACHE = {}


def _build():
    if "nc" in _NC_CACHE:
        return _NC_CACHE["nc"]
    import concourse.bacc as bacc
    import concourse.tile as tile
    import concourse.mybir as mybir
    from concourse.bass import AP

    F32 = mybir.dt.float32
    BF16 = mybir.dt.bfloat16
    AF = mybir.ActivationFunctionType
    ALU = mybir.AluOpType

    nc = bacc.Bacc("TRN2", target_bir_lowering=False, debug=False)

    xT_d = nc.dram_tensor("xT", [NCH, 128, NKT * TCH], BF16,
                          kind="ExternalInput").ap()
    wq_d = nc.dram_tensor("wq", [128, NKT * 128], BF16, kind="ExternalInput").ap()
    wk_d = nc.dram_tensor("wk", [128, NKT * 128], BF16, kind="ExternalInput").ap()
    wv_d = nc.dram_tensor("wv", [128, NKT, VW], BF16, kind="ExternalInput").ap()
    bq_d = nc.dram_tensor("bq", [128, 1], F32, kind="ExternalInput").ap()
    bk_d = nc.dram_tensor("bk", [128, 1], F32, kind="ExternalInput").ap()
    bv_d = nc.dram_tensor("bv", [1, VW], BF16, kind="ExternalInput").ap()
    wo_d = nc.dram_tensor("wo", [128, D_MODEL], BF16, kind="ExternalInput").ap()
    out_d = nc.dram_tensor("out", [BL, D_MODEL], BF16, kind="ExternalOutput").ap()

    with tile.TileContext(nc) as tc:
        with (
            tc.tile_pool(name="weights", bufs=1) as wpool,
            tc.tile_pool(name="persist", bufs=1) as ppool,
            tc.tile_pool(name="xin", bufs=NCH) as xpool,
            tc.tile_pool(name="expP", bufs=3) as epool,
            tc.tile_pool(name="oout", bufs=3) as opool,
            tc.tile_pool(name="rzP", bufs=2) as rzpool,
            tc.tile_pool(name="zrP", bufs=2) as zrpool,
            tc.tile_pool(name="dram", bufs=2, space="DRAM") as dpool,
            tc.tile_pool(name="psS", bufs=2, space="PSUM") as psS,
            tc.tile_pool(name="psPV", bufs=1, space="PSUM") as psPV,
            tc.tile_pool(name="psG", bufs=2, space="PSUM") as psG,
        ):
            # ---- static tiles ----
            wq_t = wpool.tile([128, NKT * 128], BF16, tag="wq")
            wk_t = wpool.tile([128, NKT * 128], BF16, tag="wk")
            wv_t = wpool.tile([128, NKT, VW], BF16, tag="wv")
            bq_t = wpool.tile([128, 1], F32, tag="bq")
            bk_t = wpool.tile([128, 1], F32, tag="bk")
            bv_t = wpool.tile([1, VW], BF16, tag="bv")
            wo_t = wpool.tile([128, D_MODEL], BF16, tag="wo")
            for t, d in ((wq_t, wq_d), (wk_t, wk_d), (wv_t, wv_d),
                         (bq_t, bq_d), (bk_t, bk_d), (bv_t, bv_d),
                         (wo_t, wo_d)):
                nc.gpsimd.dma_start(t[:], d)

            QT = ppool.tile([128, BL], BF16, tag="QT")
            KT = ppool.tile([128, BL], BF16, tag="KT")
            Vaug = ppool.tile([128, B * NKB, VW], BF16, tag="Vaug")
            attnU = [ppool.tile([128, L], BF16, tag=f"attnU{b}",
                                name=f"attnU{b}") for b in range(B)]
            ones_t = ppool.tile([1, 640], BF16, tag="ones")
            scr = ppool.tile([1, 32], F32, tag="scr")
            scrb = ppool.tile([1, 32], BF16, tag="scrb")
            # persistent Z-transpose scratch (single-engine reuse is safe;
            # memset once so StreamTranspose never reads uninitialized data)
            st0 = ppool.tile([96, 512], F32, tag="stp0")
            st1 = ppool.tile([96, 512], F32, tag="stp1")
            zt0 = ppool.tile([32, 512], F32, tag="zt0")
            zt1 = ppool.tile([32, 512], F32, tag="zt1")
            ztr0 = ppool.tile([32, 512], F32, tag="ztr0")
            ztr1 = ppool.tile([32, 512], F32, tag="ztr1")
            rb0 = ppool.tile([32, 512], F32, tag="rb0")
            rb1 = ppool.tile([32, 512], F32, tag="rb1")

            nc.vector.memset(ones_t[:], 1.0)
            nc.vector.memset(scr[:], 0.0)
            nc.vector.memset(st0[:], 1.0)
            nc.vector.memset(st1[:], 1.0)
            nc.vector.memset(ztr0[:], 1.0)
            nc.vector.memset(ztr1[:], 1.0)

            # x chunks: host stores x chunk-major/partition-major so each
            # 512-token chunk is one fully-contiguous [128, 8KB] DMA.
            xts = []
            for c in range(NCH):
                xt = xpool.tile([128, NKT, TCH], BF16, tag="xchunk",
                                name=f"x{c}")
                if c == 0:
                    # chunk 0 loads per k-tile: the SDMA engines round-robin
                    # across queued transfers, so one big chunk-0 DMA would
                    # finish no earlier than the whole x load; small pieces
                    # complete early and unblock the first projections.
                    for kt in range(NKT):
                        nc.sync.dma_start(xt[:, kt, :],
                                          xT_d[c][:, kt * TCH:(kt + 1) * TCH])
                else:
                    nc.sync.dma_start(
                        xt[:], xT_d[c].rearrange("p (k t) -> p k t", k=NKT))
                xts.append(xt)

            # ---- warmup: lift HAM clock gate + preload exp table ----
            # ~3.5us of continuous matmuls so the PE is at K=8/8 before
            # phase A starts; the exp preloads the ACT spline table.
            wu = psG.tile([128, 512], F32, tag="g", name="warm")
            for i in range(36):
                nc.tensor.matmul(wu[:, 0:128], ones_t[0:1, 0:128],
                                 ones_t[0:1, 128:256],
                                 start=(i == 0), stop=(i == 35))
            nc.scalar.activation(scrb[:], wu[0:1, 0:32], AF.Exp)

            # ---- emit helpers ----
            psq = {}

            def phA_qk(c, w_t, b_t, dst, nm, half):
                """Half a projection (4 k-tiles) for a 512-token chunk.

                Split so each bg thunk stays under ~1us of PE time; the two
                halves share one psum accumulator stashed in psq.
                """
                xt = xts[c]
                if half == 0:
                    ps = psq[(nm, c)] = psG.tile([128, 512], F32, tag="g",
                                                 name=f"{nm}{c}")
                else:
                    ps = psq.pop((nm, c))
                for kt in range(4 * half, 4 * half + 4):
                    nc.tensor.matmul(
                        ps[:], w_t[:, kt * 128:(kt + 1) * 128],
                        xt[:, kt, :],
                        start=(kt == 0), stop=(kt == NKT - 1),
                    )
                if half == 1:
                    nc.vector.tensor_scalar_add(
                        dst[:, c * TCH:(c + 1) * TCH], ps[:], b_t[:, 0:1])

            def phA_v(c, half):
                """Natural-layout V for 256 tokens (2 token-tiles)."""
                xt = xts[c]
                for tt in (2 * half, 2 * half + 1):
                    g = c * 4 + tt
                    vps = psG.tile([128, 512], F32, tag="g", name=f"v{g}")
                    for kt in range(NKT):
                        nc.tensor.matmul(
                            vps[:, 0:VW],
                            xt[:, kt, tt * 128:(tt + 1) * 128],
                            wv_t[:, kt, :],
                            start=(kt == 0), stop=False,
                        )
                    nc.tensor.matmul(vps[:, 0:VW], ones_t[0:1, 0:128],
                                     bv_t[:], start=False, stop=True)
                    nc.vector.tensor_copy(Vaug[:, g, :], vps[:, 0:VW])

            def sc_exp(b, qc, kt):
                """Both heads' scoresT + exp for one k-tile; returns ex."""
                q0 = b * L + qc * QC
                ksl = slice(b * L + kt * 128, b * L + (kt + 1) * 128)
                sc = psS.tile([128, 1024], F32, tag="sc")
                nc.tensor.matmul(sc[:, 0:512], KT[0:64, ksl],
                                 QT[0:64, q0:q0 + QC], start=True, stop=True)
                nc.tensor.matmul(sc[:, 512:1024], KT[64:128, ksl],
                                 QT[64:128, q0:q0 + QC], start=True, stop=True)
                ex = epool.tile([128, 1024], BF16, tag="ex")
                nc.scalar.activation(ex[:], sc[:], AF.Exp)
                return ex

            def pv_mm(b, kt, ex, pv0, pv1):
                """PV accumulate for one k-tile (lags sc_exp by 2)."""
                g = b * NKB + kt
                nc.tensor.matmul(pv0[:], Vaug[:, g, 0:DH + 1], ex[:, 0:512],
                                 start=(kt == 0), stop=(kt == NKB - 1))
                nc.tensor.matmul(pv1[:], Vaug[:, g, DH + 1:VW],
                                 ex[:, 512:1024],
                                 start=(kt == 0), stop=(kt == NKB - 1))

            def qc_evac(b, qc, pv0, pv1):
                """Stage pv out of psum fast; normalize attnU off the copy.

                1/Z without DMA round-trips: the Z row is moved onto
                partitions with a DVE 32x32 StreamTranspose, reciprocal'd on
                a strided [32,16] view (16 elems/lane), transposed back, and
                partition-broadcast into [64,512] multiplier tiles. The
                whole chain is DVE/gpsimd-internal, so the in-order DVE FIFO
                never waits on external events; only the stt pair (which
                needs the gpsimd broadcast round-trip) is deferred into the
                next window."""
                nc.vector.tensor_copy(st0[0:65, :], pv0[:])
                nc.vector.tensor_copy(st1[0:65, :], pv1[:])
                for st, zt, ztr, rb in ((st0, zt0, ztr0, rb0),
                                        (st1, zt1, ztr1, rb1)):
                    nc.vector.transpose(zt[:], st[64:96, :])
                    nc.vector.reciprocal(ztr[0:32, 0:512:32],
                                         zt[0:32, 0:512:32])
                    nc.vector.transpose(rb[:], ztr[:])
                rzm0 = rzpool.tile([64, 512], F32, tag="rzm0",
                                   name=f"rza{b}{qc}")
                rzm1 = rzpool.tile([64, 512], F32, tag="rzm1",
                                   name=f"rzb{b}{qc}")
                nc.gpsimd.partition_broadcast(rzm0[:], rb0[0:1, :])
                nc.gpsimd.partition_broadcast(rzm1[:], rb1[0:1, :])

                def part2():
                    qsl = slice(qc * QC, (qc + 1) * QC)
                    nc.vector.scalar_tensor_tensor(
                        attnU[b][0:64, qsl], st0[0:64, :], 1.0, rzm0[:],
                        op0=ALU.mult, op1=ALU.mult)
                    nc.vector.scalar_tensor_tensor(
                        attnU[b][64:128, qsl], st1[0:64, :], 1.0, rzm1[:],
                        op0=ALU.mult, op1=ALU.mult)

                return [part2]

            def op_unit(b, rc, oc):
                """Output projection for 128 tokens x 512 out-cols."""
                lsl = slice(rc * 128, (rc + 1) * 128)
                rsl = slice(b * L + rc * 128, b * L + (rc + 1) * 128)
                osl = slice(oc * 512, (oc + 1) * 512)
                ps = psG.tile([128, 512], F32, tag="g", name=f"o{b}{rc}{oc}")
                nc.tensor.matmul(ps[:], attnU[b][:, lsl], wo_t[:, osl],
                                 start=True, stop=True)
                ot = opool.tile([128, 512], BF16, tag="ot")
                nc.vector.tensor_copy(ot[:], ps[:])
                nc.sync.dma_start(out_d[rsl, osl], ot[:])

            # ---- software-pipelined emission ----
            # bg FIFO: PE slack work (phase-A halves, out-proj units)
            # consumed one thunk per kt step, placed so each chunk's K/V
            # lands before the kt that needs it and each window stays under
            # the ACT budget. QK halves must occupy ADJACENT slots (they
            # share a rotating psum accumulator).
            bg = []

            def run_bg(n=1):
                for _ in range(min(n, len(bg))):
                    bg.pop(0)()

            def qh(c, h, w_t=None, b_t=None, dst=None, nm=None):
                if w_t is None:
                    w_t, b_t, dst, nm = wq_t, bq_t, QT, "q"
                return lambda: phA_qk(c, w_t, b_t, dst, nm, h)

            def kh(c, h):
                return qh(c, h, wk_t, bk_t, KT, "k")

            def vh(c, h):
                return lambda: phA_v(c, h)

            # fast start: Q/K of chunk 0 inline (unsplit emission order)
            phA_qk(0, wq_t, bq_t, QT, "q", 0)
            phA_qk(0, wq_t, bq_t, QT, "q", 1)
            phA_qk(0, wk_t, bk_t, KT, "k", 0)
            phA_qk(0, wk_t, bk_t, KT, "k", 1)
            def filler():
                """Dep-free matmuls: keep the HAM clock gate open in
                underloaded windows."""
                f = psG.tile([128, 512], F32, tag="g", name="fill")
                for i in range(3):
                    nc.tensor.matmul(f[:], ones_t[0:1, 0:128],
                                     ones_t[0:1, 128:640],
                                     start=(i == 0), stop=(i == 2))

            # per-window background schedules (deadline-ordered; b0-qc0 is
            # structurally oversubscribed, later b0 windows carry one b1
            # chunk each, b1 windows get Q-only plus fillers)
            sched = {
                (0, 0): [vh(0, 0), vh(0, 1), kh(1, 0), kh(1, 1),
                         vh(1, 0), vh(1, 1), kh(2, 0), kh(2, 1),
                         vh(2, 0), vh(2, 1), kh(3, 0), kh(3, 1),
                         vh(3, 0), vh(3, 1), qh(1, 0), qh(1, 1)],
                (0, 1): [qh(2, 0), qh(2, 1), kh(4, 0), kh(4, 1),
                         vh(4, 0), vh(4, 1)],
                (0, 2): [qh(3, 0), qh(3, 1), kh(5, 0), kh(5, 1),
                         vh(5, 0), vh(5, 1)],
                (0, 3): [qh(4, 0), qh(4, 1), kh(6, 0), kh(6, 1),
                         vh(6, 0), vh(6, 1)],
                (1, 0): [qh(5, 0), qh(5, 1), kh(7, 0), kh(7, 1),
                         vh(7, 0), vh(7, 1)],
                (1, 1): [qh(6, 0), qh(6, 1)],
                (1, 2): [qh(7, 0), qh(7, 1)],
            }

            # out-proj units carry their source-window index: a unit's
            # attnU arrives ~8us after its window ends (the Z chain), and
            # the PE runs ~2 exps ahead, so a unit is only eligible two
            # windows later, or in the last quarter of the next window.
            opq = []
            widx = 0

            def pop_op(slot):
                for j, (ws, th) in enumerate(opq):
                    if widx - ws >= 2 or (widx - ws == 1 and slot >= 8):
                        opq.pop(j)
                        return th
                return None

            deferred = []
            for b in range(B):
                for qc in range(NQC):
                    bg = sched.get((b, qc), []) + bg
                    pv0 = psPV.tile([65, 512], F32, tag="pv0",
                                    name=f"pv0_{b}{qc}")
                    pv1 = psPV.tile([65, 512], F32, tag="pv1",
                                    name=f"pv1_{b}{qc}")
                    exq = {}
                    for kt in range(NKB):
                        exq[kt] = sc_exp(b, qc, kt)
                        if kt >= 2:
                            pv_mm(b, kt - 2, exq.pop(kt - 2), pv0, pv1)
                        # previous window's deferred DVE chain pieces: by
                        # kt=4/8 their DMA inputs are long resident, so the
                        # in-order DVE FIFO never stalls on them
                        if kt == 4 and deferred:
                            deferred.pop(0)()
                        if kt == 8 and deferred:
                            deferred.pop(0)()
                        th = None
                        if kt >= 8 or not bg:
                            th = pop_op(kt)
                        if th is not None:
                            th()
                        elif bg:
                            run_bg(1)
                    pv_mm(b, NKB - 2, exq.pop(NKB - 2), pv0, pv1)
                    pv_mm(b, NKB - 1, exq.pop(NKB - 1), pv0, pv1)
                    deferred += qc_evac(b, qc, pv0, pv1)
                    for rc in range(qc * 4, (qc + 1) * 4):
                        for oc in range(2):
                            opq.append((widx, lambda b=b, rc=rc, oc=oc:
                                        op_unit(b, rc, oc)))
                    widx += 1
            # tail: resolve the last evac chain, then drain remaining units
            widx += 1
            while deferred:
                deferred.pop(0)()
            while opq:
                opq.pop(0)[1]()

    nc.compile()
    _NC_CACHE["nc"] = nc
    return nc


def _shard_inputs(x, W_qkv, b_qkv, W_o):
    import ml_dtypes
    BF = ml_dtypes.bfloat16
    # [NCH, 128, NKT*TCH]: xT[c, p, kt*TCH+t] = x[c*TCH+t, kt*128+p]
    xT = np.ascontiguousarray(
        x.reshape(NCH, TCH, NKT, 128).transpose(0, 3, 2, 1)
        .reshape(NCH, 128, NKT * TCH).astype(BF))

    def lhsT_layout(w):
        # [D_MODEL, 128] -> [128, NKT*128] with [p, kt*128+ch] = w[kt*128+p, ch]
        return np.ascontiguousarray(
            w.reshape(NKT, 128, 128).transpose(1, 0, 2)
            .reshape(128, NKT * 128).astype(BF))

    in_maps = []
    for c in range(NCORES):
        cs = slice(c * 128, (c + 1) * 128)
        wq = W_qkv[:, cs] * 0.125
        wk = W_qkv[:, D_MODEL:][:, cs]
        wv = W_qkv[:, 2 * D_MODEL:][:, cs]
        # Wv_aug: [V_h0 | 0 | V_h1 | 0] columns; bias row carries [bv_h0 | 1
        # | bv_h1 | 1] so the ones-row matmul bakes both bias and the softmax
        # ones-columns.
        wv_aug = np.zeros((D_MODEL, VW), dtype=np.float32)
        wv_aug[:, 0:DH] = wv[:, 0:DH]
        wv_aug[:, DH + 1:2 * DH + 1] = wv[:, DH:2 * DH]
        bv = b_qkv[2 * D_MODEL:][cs]
        bv_aug = np.zeros((VW,), dtype=np.float32)
        bv_aug[0:DH] = bv[0:DH]
        bv_aug[DH] = 1.0
        bv_aug[DH + 1:2 * DH + 1] = bv[DH:2 * DH]
        bv_aug[VW - 1] = 1.0
        in_maps.append({
            "xT": xT,
            "wq": lhsT_layout(wq), "wk": lhsT_layout(wk),
            "wv": np.ascontiguousarray(
                wv_aug.reshape(NKT, 128, VW).transpose(1, 0, 2).astype(BF)),
            "bq": np.ascontiguousarray(
                b_qkv[cs] * 0.125, dtype=np.float32).reshape(128, 1),
            "bk": np.ascontiguousarray(
                b_qkv[D_MODEL:][cs], dtype=np.float32).reshape(128, 1),
            "bv": np.ascontiguousarray(bv_aug.astype(BF)).reshape(1, VW),
            "wo": np.ascontiguousarray(W_o[cs, :].astype(BF)),
        })
    return in_maps


def _run(inputs, trace=False, tmpdir=None):
    from concourse.bass_utils import run_bass_kernel_spmd

    _register_ntff_hook()
    nc = _build()
    in_maps = _shard_inputs(
        np.asarray(inputs["x"], dtype=np.float32),
        np.asarray(inputs["W_qkv"], dtype=np.float32),
        np.asarray(inputs["b_qkv"], dtype=np.float32),
        np.asarray(inputs["W_o"], dtype=np.float32),
    )
    res = run_bass_kernel_spmd(nc, in_maps, core_ids=list(range(NCORES)),
                               trace=trace, tmpdir=tmpdir)
    partial = np.zeros((BL, D_MODEL), dtype=np.float32)
    for c in range(NCORES):
        partial += res.results[c]["out"].astype(np.float32)
    out = (partial + np.asarray(inputs["b_o"], dtype=np.float32))
    return out.astype(np.float32).reshape(B, L, D_MODEL), res


def kernel(**inputs) -> np.ndarray:
    out, _ = _run(inputs, trace=False)
    return out
